# revision 48
# baseline (speedup 1.0000x reference)
"""GAT (5-layer, dense-adjacency) Trainium2 kernel, sharded across 8 NeuronCores.

Sharding: query-node rows split 512/core. Per layer each core computes its
own row-block of the augmented projection [Wh | ones | d] (the ones column
yields softmax denominators straight out of the attention matmul) and
AllGathers it in fp32r per head-group — group 0's payload also carries every
head's d column, so the scores (which only need d) unblock on the first,
smallest gather while later groups' gathers hide under earlier groups'
compute. Scores: fused DVE op (leaky(s+d)+addmask, bf16 additive mask built
on-chip via PE transposes) + ACT exp + one fp32r (TF32) matmul per
(head, j-tile) at bf16 speed. Softmax division, ELU and the final mean-pool
partial reduction run split across DVE/PE/Pool/ACT so the inter-layer
critical path stays short; the [8]-vector partials are combined host-side.
"""

import numpy as np

import concourse.bacc as bacc
import concourse.mybir as mybir
import concourse.tile as tile
from concourse.bass_utils import run_bass_kernel_spmd

import concourse.dve_ops as dve_ops
from concourse.dve_spec import Spec, Src0, Src1, C0, C1, maxx, lower
from concourse.dve_spec import _has_src1 as _spec_has_src1
from concourse.dve_uop import DveOpSpec

try:
    import ml_dtypes

    _BF16 = ml_dtypes.bfloat16
except ImportError:  # pragma: no cover
    _BF16 = np.float32

dt = mybir.dt
AF = mybir.ActivationFunctionType

# ---------------------------------------------------------------- constants
N = 4096
NCORE = 8
ROWS = N // NCORE  # 512 query rows per core
P = 128
JT = N // P  # 32 j-tiles
NEG = -30000.0  # additive mask for non-edges; exp(x-30000) == 0
ALPHA = 0.1
# (fin, fout, heads, concat, elu_after, group_size)
CFG = [
    (256, 128, 8, True, True, 4),
    (128, 64, 8, True, True, 4),
    (64, 32, 4, True, True, 2),
    (32, 16, 1, True, False, 1),
    (16, 8, 1, False, False, 1),
]

# ---------------------------------------------------------------- custom op
LEAKY_BIAS_ADDMASK = dve_ops.DveOp(
    "LEAKY_BIAS_ADDMASK",
    Spec(
        body=maxx(Src0 + C0, (Src0 + C0) * C1) + Src1,
        reference=lambda in0, in1, s0, s1, imm2: (
            np.maximum(in0 + s0, (in0 + s0) * s1) + in1
        ).astype(np.float32),
    ),
    subdim=False,
    uops_sha={},
)


def _register_custom_op(op):
    if op.name in dve_ops._SUB_OPCODE_FOR_NAME:
        return
    idx = dve_ops._CUSTOM_DVE_ROW_BASE + len(dve_ops.OPS)
    assert idx < 0x20
    dve_ops.OPS.append(op)
    dve_ops.CUSTOM_DVE_SPECS[op.name] = op.spec
    dve_ops._SUB_OPCODE_FOR_NAME[op.name] = idx
    shas = {}
    for ver in ("v3", "v4"):
        try:
            s = DveOpSpec(
                name=op.name,
                opcode=idx,
                uops=lower(op.spec, ver=ver),
                rd1_en=_spec_has_src1(op.spec),
            )
            shas[ver] = s.sha(ver)
        except Exception:
            pass
    object.__setattr__(op, "uops_sha", shas)


_register_custom_op(LEAKY_BIAS_ADDMASK)


def _groups(h, g):
    return [list(range(g0, min(g0 + g, h))) for g0 in range(0, h, g)]


# ---------------------------------------------------------------- builder
def build_kernel():
    import os as _os

    debug_taps = bool(_os.environ.get("DEBUG_TAPS"))
    nc = bacc.Bacc("TRN2", target_bir_lowering=False, debug=False)

    adjrows = nc.dram_tensor("adjrows", [ROWS, N], dt.int32, kind="ExternalInput")
    x0T_own = nc.dram_tensor("x0T_own", [256, ROWS], dt.float32, kind="ExternalInput")
    wext_dram = {}
    ws_dram = {}
    for li, (fin, fout, h, concat, _elu, _g) in enumerate(CFG, start=1):
        dh = fout // h if concat else fout
        cw2 = h * (dh + 1) + h  # per-head [values | 0(->1)] blocks, then d cols
        wext_dram[li] = nc.dram_tensor(
            f"wext{li}", [fin, cw2], dt.float32, kind="ExternalInput"
        )
        ws_dram[li] = nc.dram_tensor(f"ws{li}", [fin, h], dt.float32, kind="ExternalInput")

    pool_out = nc.dram_tensor("pool_part", [8, 1], dt.float32, kind="ExternalOutput")
    if debug_taps:
        dbg_x = {}
        for _li, (_f, _fo, _h, _c, _e, _g) in enumerate(CFG, start=1):
            dbg_x[_li] = nc.dram_tensor(
                f"dbg_x{_li}", [_fo, ROWS], dt.float32, kind="ExternalOutput"
            )

    ident_np = np.eye(P, dtype=_BF16)
    ident_dram = nc.inline_tensor(ident_np, name="ident128")

    with tile.TileContext(nc) as tc:
        with (
            tc.tile_pool(name="persist", bufs=1) as persist,
            tc.tile_pool(name="dram", bufs=1, space="DRAM") as drampool,
            tc.tile_pool(name="xTown", bufs=3) as xTown_pool,
            tc.tile_pool(name="layerbuf", bufs=2) as layerbuf,
            tc.tile_pool(name="ownp", bufs=2) as ownp,
            tc.tile_pool(name="srep", bufs=1) as srep_pool,
            tc.tile_pool(name="work", bufs=2) as work,
            tc.tile_pool(name="pjt", bufs=4) as pjt_pool,
            tc.tile_pool(name="pjts", bufs=5) as pjts_pool,
            tc.tile_pool(name="small", bufs=2) as small,
            tc.tile_pool(name="whps", bufs=1, space="PSUM") as whps,
            tc.tile_pool(name="sps", bufs=1, space="PSUM") as sps,
            tc.tile_pool(name="trps", bufs=1, space="PSUM") as trps,
            tc.tile_pool(name="attps", bufs=4, space="PSUM") as attps,
        ):
            # ---------------- persistent tiles
            maskT = persist.tile([P, JT, ROWS], dt.bfloat16, tag="maskT")
            ident_sb = persist.tile([P, P], dt.bfloat16, tag="ident")
            nc.sync.dma_start(ident_sb[:], ident_dram[:])
            ones_row = persist.tile([1, P], dt.float32, tag="ones_row")
            nc.vector.memset(ones_row[:], 1.0)
            ones_blk = persist.tile([P, 32], dt.float32, tag="ones_blk")
            nc.vector.memset(ones_blk[:], 1.0)
            negb = persist.tile([P, 1], dt.float32, tag="negb")
            nc.vector.memset(negb[:], NEG)

            wext_sb = {}
            ws_sb = {}
            for li, (fin, fout, h, concat, _elu, _g) in enumerate(CFG, start=1):
                dh = fout // h if concat else fout
                cw2 = h * (dh + 1) + h
                nft = (fin + P - 1) // P
                wext_sb[li] = []
                ws_sb[li] = []
                for ft in range(nft):
                    fr = min(P, fin - ft * P)
                    wt = persist.tile([fr, cw2], dt.float32, tag=f"wext{li}_{ft}")
                    nc.sync.dma_start(wt[:], wext_dram[li][ft * P : ft * P + fr, :])
                    wext_sb[li].append(wt)
                    st = persist.tile([fr, h], dt.float32, tag=f"ws{li}_{ft}")
                    nc.sync.dma_start(st[:], ws_dram[li][ft * P : ft * P + fr, :])
                    ws_sb[li].append(st)

            # ---------------- L1 own activations from input
            xTown_cur = []
            for ft in range(2):
                to = xTown_pool.tile([P, ROWS], dt.float32, tag="xTown")
                nc.sync.dma_start(to[:], x0T_own[ft * P : (ft + 1) * P, :])
                xTown_cur.append(to)

            def build_mask(chunks):
                # transpose adj rows -> additive maskT (bf16).
                CH = 1024
                for c0 in chunks:
                    for ib in range(ROWS // P):
                        stage_i = work.tile([P, CH], dt.int32, tag="stage_i")
                        # ACT's DGE queue: keeps this bulk stream from blocking
                        # the latency-critical gather/unpack DMAs on SP's queue
                        nc.scalar.dma_start(
                            stage_i[:], adjrows[ib * P : (ib + 1) * P, c0 : c0 + CH]
                        )
                        stage_b = work.tile([P, CH], dt.bfloat16, tag="stage_b")
                        nc.gpsimd.tensor_copy(stage_b[:], stage_i[:])
                        for k in range(CH // P):
                            jt = (c0 + k * P) // P
                            tps = trps.tile([P, P], dt.bfloat16, tag="tps")
                            nc.tensor.transpose(
                                tps[:], stage_b[:, k * P : (k + 1) * P], ident_sb[:]
                            )
                            # adj -> additive mask: adj*(-NEG) + NEG, on
                            # ACT (idle at startup) to keep the DVE bulk lean
                            nc.scalar.activation(
                                maskT[:, jt, ib * P : (ib + 1) * P],
                                tps[:],
                                AF.Identity,
                                bias=negb[:],
                                scale=-NEG,
                            )

            for li, (fin, fout, h, concat, elu, G) in enumerate(CFG, start=1):
                dh = fout // h if concat else fout
                dh1 = dh + 1
                cw2 = h * dh1 + h
                nft = (fin + P - 1) // P
                is_last = li == len(CFG)
                groups = _groups(h, G)

                # ---- own-block [Wh+ones (group-major) | all d cols].
                # The tiny d-only AllGather launches FIRST (scores gate on it);
                # vals AllGathers follow per group (matmuls gate on those).
                pw_all = ownp.tile([P, 4, cw2], dt.float32r, tag="own_sb")
                for k in range(4):
                    pw = whps.tile([P, cw2], dt.float32, tag="pw")
                    for ft in range(nft):
                        fr = min(P, fin - ft * P)
                        nc.tensor.matmul(
                            pw[:],
                            xTown_cur[ft][0:fr, k * P : (k + 1) * P],
                            wext_sb[li][ft][:],
                            start=(ft == 0),
                            stop=(ft == nft - 1),
                        )
                    nc.scalar.copy(pw_all[:, k, :], pw[:])
                # full-width gathers split along own-row halves: every
                # head's vals+d for rows k01 land in half the wire time;
                # the k23 half's lag is covered by k01 score work
                vals = pw_all[:, :, 0 : h * dh1].rearrange(
                    "p k (a b) -> p k a b", a=h
                )
                nc.scalar.copy(
                    vals[:, :, :, dh : dh + 1],
                    ones_blk[:, 0 : 4 * h].rearrange(
                        "p (k a b) -> p k a b", k=4, a=h
                    ),
                )
                halves = [(0, 4)]  # one gather per layer: real HW charges far more per collective than the cost model
                ag_halves = []
                for hi_, (k0, k1) in enumerate(halves):
                    nk = k1 - k0
                    ag_in = drampool.tile(
                        [nk * P, cw2],
                        dt.float32r,
                        tag=f"agin{li}h{hi_}",
                        name=f"agi{hi_}",
                    )
                    ag_out = drampool.tile(
                        [NCORE, nk * P, cw2],
                        dt.float32r,
                        tag=f"agout{li}h{hi_}",
                        addr_space="Shared",
                        name=f"ago{hi_}",
                    )
                    nc.sync.dma_start(
                        ag_in.rearrange("(k p) c -> p k c", p=P),
                        pw_all[:, k0:k1, :],
                    )
                    nc.gpsimd.collective_compute(
                        "AllGather",
                        mybir.AluOpType.bypass,
                        replica_groups=[list(range(NCORE))],
                        ins=[ag_in.opt()],
                        outs=[ag_out.opt()],
                    )
                    ag_halves.append(ag_out)

                # ---- unpack: one DMA per (half, k) into jt-strided slices
                whrow = layerbuf.tile(
                    [P, JT, cw2], dt.float32r, tag="whrow0", name="wr0"
                )
                wr0_4 = whrow.rearrange("p (r k) c -> p r k c", k=4)
                for hi_, (k0, k1) in enumerate(halves):
                    for k2 in range(k1 - k0):
                        nc.sync.dma_start(
                            wr0_4[:, :, k0 + k2, :],
                            ag_halves[hi_].rearrange(
                                "r (k p) c -> p r k c", p=P
                            )[:, :, k2, :],
                        )
                whrow_g = [whrow] * len(groups)
                d_all = whrow.rearrange("p j c -> p j c")[
                    :, :, h * dh1 : cw2
                ].bitcast(dt.float32)
                d_g = [d_all[:, :, gs[0] : gs[0] + len(gs)] for gs in groups]

                mask_chunks = list(range(0, N, 1024))

                def emit_sreps(hhs):
                    for hh in hhs:
                        ps_row = sps.tile([1, ROWS], dt.float32, tag="ps_row")
                        for ft in range(nft):
                            fr = min(P, fin - ft * P)
                            nc.tensor.matmul(
                                ps_row[:],
                                ws_sb[li][ft][:, hh : hh + 1],
                                xTown_cur[ft][0:fr, :],
                                start=(ft == 0),
                                stop=(ft == nft - 1),
                            )
                        s_row = small.tile([1, ROWS], dt.float32, tag="vec1")
                        nc.scalar.copy(s_row[:], ps_row[:])
                        srt = srep_pool.tile(
                            [P, ROWS], dt.float32, tag=f"srep{hh}", name=f"sr{hh}"
                        )
                        nc.gpsimd.partition_broadcast(srt[:], s_row[:])
                        sreps[hh] = srt

                sreps = {}
                if li == 1:
                    build_mask(mask_chunks[0:1])
                    mask_built = {0}
                    emit_sreps(range(h))
                else:
                    mask_built = set(range(len(mask_chunks)))
                    emit_sreps(range(h))

                # ---- attention per head group
                xnext = xTown_pool.tile([fout, ROWS], dt.float32, tag="xTown")
                n_groups = len(groups)
                # j-tiles whose group-0 gather half landed first go first
                if len(halves) == 2:
                    jt_order = [j for j in range(JT) if j % 4 < 2] + [
                        j for j in range(JT) if j % 4 >= 2
                    ]
                else:
                    jt_order = list(range(JT))

                def emit_epi2(gi, gs, osbs):
                    last_g = gi == n_groups - 1
                    for k, hh in enumerate(gs):
                        dve_path = last_g and (k % 2 == 0)
                        o_sb = osbs[hh]
                        # compute engines can't read at partition offset dh
                        # (must be 0/32/64/96); DMA the denominator row down
                        r_in = small.tile(
                            [1, ROWS], dt.float32, tag="vec1i", name="r_in"
                        )
                        nc.sync.dma_start(r_in[:], o_sb[dh : dh + 1, :])
                        r_sb = small.tile(
                            [1, ROWS], dt.float32, tag="vec1", name="r_sb"
                        )
                        nc.vector.reciprocal(r_sb[:], r_in[:])
                        ohead = small.tile(
                            [dh, ROWS], dt.float32, tag="ohead", name="ohead"
                        )
                        if dve_path:
                            rps = trps.tile([dh, ROWS], dt.float32, tag="rps")
                            nc.tensor.matmul(
                                rps[:], ones_row[0:1, 0:dh], r_sb[:],
                                start=True, stop=True,
                            )
                            nc.vector.tensor_mul(ohead[:], o_sb[0:dh, :], rps[:])
                        else:
                            rrep = small.tile(
                                [dh, ROWS], dt.float32, tag="rrep", name="rrep"
                            )
                            nc.gpsimd.partition_broadcast(rrep[:], r_sb[:])
                            nc.gpsimd.tensor_mul(ohead[:], o_sb[0:dh, :], rrep[:])
                        if elu:
                            # elu(x) = max(x,0) - 1 + exp(min(x,0))
                            mmin = small.tile(
                                [dh, ROWS], dt.float32, tag="tmp1", name="mmin"
                            )
                            emin = small.tile(
                                [dh, ROWS], dt.float32, tag="tmp2", name="emin"
                            )
                            rmax = small.tile(
                                [dh, ROWS], dt.float32, tag="tmp1", name="rmax"
                            )
                            if dve_path:
                                nc.vector.tensor_scalar(
                                    mmin[:], ohead[:], 0.0, None, mybir.AluOpType.min
                                )
                                nc.scalar.activation(emin[:], mmin[:], AF.Exp)
                                nc.vector.tensor_scalar(
                                    rmax[:], ohead[:], 0.0, -1.0,
                                    mybir.AluOpType.max, mybir.AluOpType.add,
                                )
                                nc.vector.tensor_add(ohead[:], rmax[:], emin[:])
                            else:
                                nc.gpsimd.tensor_scalar(
                                    mmin[:], ohead[:], 0.0, None, mybir.AluOpType.min
                                )
                                nc.scalar.activation(emin[:], mmin[:], AF.Exp)
                                nc.gpsimd.tensor_scalar(
                                    rmax[:], ohead[:], 0.0, -1.0,
                                    mybir.AluOpType.max, mybir.AluOpType.add,
                                )
                                nc.gpsimd.tensor_add(ohead[:], rmax[:], emin[:])
                        nc.sync.dma_start(xnext[hh * dh : (hh + 1) * dh, :], ohead[:])

                pending_epi = None
                att_accs = []
                for gi, gs in enumerate(groups):
                    ng = len(gs)
                    last_group = gi == n_groups - 1
                    att_acc = {}
                    for hh in gs:
                        att_acc[hh] = attps.tile(
                            [dh1, ROWS], dt.float32, tag="att", name=f"att{hh}"
                        )
                    att_accs.append(att_acc)
                    for jx, jt in enumerate(jt_order):
                        if jx == 8 and pending_epi is not None:
                            emit_epi2(*pending_epi)
                            pending_epi = None
                        ci = jt // 8
                        if ci not in mask_built:
                            build_mask(mask_chunks[ci : ci + 1])
                            mask_built.add(ci)
                        l_jt = work.tile([P, ng * ROWS], dt.float32, tag="l_jt")
                        for k, hh in enumerate(gs):
                            nc.vector._custom_dve(
                                LEAKY_BIAS_ADDMASK,
                                out=l_jt[:, k * ROWS : (k + 1) * ROWS],
                                in0=sreps[hh][:],
                                in1=maskT[:, jt, :],
                                s0=d_g[gi][:, jt, k : k + 1],
                                s1=ALPHA,
                            )
                        pool_p = pjts_pool if h == 1 else pjt_pool
                        p_jt = pool_p.tile([P, ng * ROWS], dt.float32r, tag="p_jt")
                        nc.scalar.activation(p_jt[:], l_jt[:], AF.Exp)
                        for k, hh in enumerate(gs):
                            nc.tensor.matmul(
                                att_acc[hh][:],
                                whrow_g[gi][:, jt, hh * dh1 : (hh + 1) * dh1],
                                p_jt[:, k * ROWS : (k + 1) * ROWS],
                                start=(jx == 0),
                                stop=(jx == JT - 1),
                            )
                    # epilogue phase 1: copy PSUM out now (frees att banks for
                    # the next group); phase 2 is deferred for non-last groups
                    # until 8 j-tiles into the next group's emission so its
                    # dependency waits never stall the in-order engine queues
                    osbs = {}
                    for k, hh in enumerate(gs):
                        o_sb = small.tile(
                            [dh1, ROWS], dt.float32, tag=f"o_sb{k}", name=f"os{k}"
                        )
                        nc.scalar.copy(o_sb[:], att_acc[hh][:])
                        osbs[hh] = o_sb
                    if last_group:
                        if pending_epi is not None:
                            emit_epi2(*pending_epi)
                            pending_epi = None
                        emit_epi2(gi, gs, osbs)
                    else:
                        pending_epi = (gi, gs, osbs)

                if debug_taps:
                    nc.sync.dma_start(dbg_x[li][:], xnext[:])
                if is_last:
                    psum_final = small.tile([fout, 1], dt.float32, tag="vec1f")
                    nc.vector.reduce_sum(
                        psum_final[:], xnext[:], axis=mybir.AxisListType.X
                    )
                    nc.sync.dma_start(pool_out[:], psum_final[:])
                else:
                    xTown_cur = [xnext]

    nc.finalize()
    return nc


_NC_CACHE = None
_last_in_maps = None


def kernel(**inputs):
    global _NC_CACHE
    node_features = np.asarray(inputs["node_features"], dtype=np.float32)
    adj = np.ascontiguousarray(np.asarray(inputs["adj_mat"], dtype=np.int32))
    fc_w = np.asarray(inputs["fc_w"], dtype=np.float32)
    fc_b = np.asarray(inputs["fc_b"], dtype=np.float32)

    x0T = node_features.T  # [256, N]

    wext = {}
    ws = {}
    for li, (fin, fout, h, concat, _elu, _g) in enumerate(CFG, start=1):
        dh = fout // h if concat else fout
        W = np.asarray(inputs[f"W{li}"], dtype=np.float32)  # [h, fin, dh]
        a_src = np.asarray(inputs[f"a_src{li}"], dtype=np.float32)  # [h, dh]
        a_dst = np.asarray(inputs[f"a_dst{li}"], dtype=np.float32)
        wd = np.einsum("hfd,hd->fh", W, a_dst).astype(np.float32)  # [fin, h]
        # augmented: per-head [W_h | zero(->ones)] blocks, then all d cols
        waug = np.zeros((fin, h * (dh + 1) + h), dtype=np.float32)
        for hh in range(h):
            waug[:, hh * (dh + 1) : hh * (dh + 1) + dh] = W[hh].reshape(fin, dh)
        waug[:, h * (dh + 1) :] = wd
        wext[li] = np.ascontiguousarray(waug)
        ws[li] = np.ascontiguousarray(
            np.einsum("hfd,hd->fh", W, a_src).astype(np.float32)
        )

    in_maps = []
    for c in range(NCORE):
        m = {
            "adjrows": np.ascontiguousarray(adj[c * ROWS : (c + 1) * ROWS, :]),
            "x0T_own": np.ascontiguousarray(x0T[:, c * ROWS : (c + 1) * ROWS]),
        }
        for li in range(1, 6):
            m[f"wext{li}"] = wext[li]
            m[f"ws{li}"] = ws[li]
        in_maps.append(m)

    if _NC_CACHE is None:
        _NC_CACHE = build_kernel()
    nc = _NC_CACHE
    global _last_in_maps
    _last_in_maps = in_maps

    res = run_bass_kernel_spmd(nc, in_maps, list(range(NCORE)))
    total = np.zeros((8,), dtype=np.float32)
    for c in range(NCORE):
        total += res.results[c]["pool_part"][:, 0]
    pooled = total / np.float32(N)
    out = pooled @ fc_w + fc_b
    return out.astype(np.float32)


# revision 49
# speedup vs baseline: 1.0297x; 1.0297x over previous
"""GAT (5-layer, dense-adjacency) Trainium2 kernel, sharded across 8 NeuronCores.

Sharding: query-node rows split 512/core. Per layer each core computes its
own row-block of the augmented projection [Wh | ones | d] (the ones column
yields softmax denominators straight out of the attention matmul) and
AllGathers it in fp32r per head-group — group 0's payload also carries every
head's d column, so the scores (which only need d) unblock on the first,
smallest gather while later groups' gathers hide under earlier groups'
compute. Scores: fused DVE op (leaky(s+d)+addmask, bf16 additive mask built
on-chip via PE transposes) + ACT exp + one fp32r (TF32) matmul per
(head, j-tile) at bf16 speed. Softmax division, ELU and the final mean-pool
partial reduction run split across DVE/PE/Pool/ACT so the inter-layer
critical path stays short; the [8]-vector partials are combined host-side.
"""

import numpy as np

import concourse.bacc as bacc
import concourse.mybir as mybir
import concourse.tile as tile
from concourse.bass_utils import run_bass_kernel_spmd

import concourse.dve_ops as dve_ops
from concourse.dve_spec import Spec, Src0, Src1, C0, C1, maxx, lower
from concourse.dve_spec import _has_src1 as _spec_has_src1
from concourse.dve_uop import DveOpSpec

try:
    import ml_dtypes

    _BF16 = ml_dtypes.bfloat16
except ImportError:  # pragma: no cover
    _BF16 = np.float32

dt = mybir.dt
AF = mybir.ActivationFunctionType

# ---------------------------------------------------------------- constants
N = 4096
NCORE = 8
ROWS = N // NCORE  # 512 query rows per core
P = 128
JT = N // P  # 32 j-tiles
NEG = -30000.0  # additive mask for non-edges; exp(x-30000) == 0
ALPHA = 0.1
# (fin, fout, heads, concat, elu_after, group_size)
CFG = [
    (256, 128, 8, True, True, 4),
    (128, 64, 8, True, True, 4),
    (64, 32, 4, True, True, 2),
    (32, 16, 1, True, False, 1),
    (16, 8, 1, False, False, 1),
]

# ---------------------------------------------------------------- custom op
LEAKY_BIAS_ADDMASK = dve_ops.DveOp(
    "LEAKY_BIAS_ADDMASK",
    Spec(
        body=maxx(Src0 + C0, (Src0 + C0) * C1) + Src1,
        reference=lambda in0, in1, s0, s1, imm2: (
            np.maximum(in0 + s0, (in0 + s0) * s1) + in1
        ).astype(np.float32),
    ),
    subdim=False,
    uops_sha={},
)


def _register_custom_op(op):
    if op.name in dve_ops._SUB_OPCODE_FOR_NAME:
        return
    idx = dve_ops._CUSTOM_DVE_ROW_BASE + len(dve_ops.OPS)
    assert idx < 0x20
    dve_ops.OPS.append(op)
    dve_ops.CUSTOM_DVE_SPECS[op.name] = op.spec
    dve_ops._SUB_OPCODE_FOR_NAME[op.name] = idx
    shas = {}
    for ver in ("v3", "v4"):
        try:
            s = DveOpSpec(
                name=op.name,
                opcode=idx,
                uops=lower(op.spec, ver=ver),
                rd1_en=_spec_has_src1(op.spec),
            )
            shas[ver] = s.sha(ver)
        except Exception:
            pass
    object.__setattr__(op, "uops_sha", shas)


_register_custom_op(LEAKY_BIAS_ADDMASK)


def _groups(h, g):
    return [list(range(g0, min(g0 + g, h))) for g0 in range(0, h, g)]


# ---------------------------------------------------------------- builder
def build_kernel():
    import os as _os

    debug_taps = bool(_os.environ.get("DEBUG_TAPS"))
    nc = bacc.Bacc("TRN2", target_bir_lowering=False, debug=False)

    adjrows = nc.dram_tensor("adjrows", [ROWS, N], dt.int32, kind="ExternalInput")
    x0T_own = nc.dram_tensor("x0T_own", [256, ROWS], dt.float32, kind="ExternalInput")
    wext_dram = {}
    ws_dram = {}
    for li, (fin, fout, h, concat, _elu, _g) in enumerate(CFG, start=1):
        dh = fout // h if concat else fout
        cw2 = h * (dh + 1) + h  # per-head [values | 0(->1)] blocks, then d cols
        wext_dram[li] = nc.dram_tensor(
            f"wext{li}", [fin, cw2], dt.float32, kind="ExternalInput"
        )
        ws_dram[li] = nc.dram_tensor(f"ws{li}", [fin, h], dt.float32, kind="ExternalInput")

    pool_out = nc.dram_tensor("pool_part", [8, 1], dt.float32, kind="ExternalOutput")
    if debug_taps:
        dbg_x = {}
        for _li, (_f, _fo, _h, _c, _e, _g) in enumerate(CFG, start=1):
            dbg_x[_li] = nc.dram_tensor(
                f"dbg_x{_li}", [_fo, ROWS], dt.float32, kind="ExternalOutput"
            )

    ident_np = np.eye(P, dtype=_BF16)
    ident_dram = nc.inline_tensor(ident_np, name="ident128")

    with tile.TileContext(nc) as tc:
        with (
            tc.tile_pool(name="persist", bufs=1) as persist,
            tc.tile_pool(name="dram", bufs=1, space="DRAM") as drampool,
            tc.tile_pool(name="xTown", bufs=3) as xTown_pool,
            tc.tile_pool(name="layerbuf", bufs=2) as layerbuf,
            tc.tile_pool(name="ownp", bufs=2) as ownp,
            tc.tile_pool(name="srep", bufs=1) as srep_pool,
            tc.tile_pool(name="work", bufs=2) as work,
            tc.tile_pool(name="pjt", bufs=4) as pjt_pool,
            tc.tile_pool(name="pjts", bufs=5) as pjts_pool,
            tc.tile_pool(name="small", bufs=2) as small,
            tc.tile_pool(name="whps", bufs=1, space="PSUM") as whps,
            tc.tile_pool(name="sps", bufs=1, space="PSUM") as sps,
            tc.tile_pool(name="trps", bufs=1, space="PSUM") as trps,
            tc.tile_pool(name="attps", bufs=4, space="PSUM") as attps,
        ):
            # ---------------- persistent tiles
            maskT = persist.tile([P, JT, ROWS], dt.bfloat16, tag="maskT")
            ident_sb = persist.tile([P, P], dt.bfloat16, tag="ident")
            nc.sync.dma_start(ident_sb[:], ident_dram[:])
            ones_row = persist.tile([1, P], dt.float32, tag="ones_row")
            nc.vector.memset(ones_row[:], 1.0)
            ones_blk = persist.tile([P, 32], dt.float32, tag="ones_blk")
            nc.vector.memset(ones_blk[:], 1.0)
            negb = persist.tile([P, 1], dt.float32, tag="negb")
            nc.vector.memset(negb[:], NEG)

            wext_sb = {}
            ws_sb = {}
            for li, (fin, fout, h, concat, _elu, _g) in enumerate(CFG, start=1):
                dh = fout // h if concat else fout
                cw2 = h * (dh + 1) + h
                nft = (fin + P - 1) // P
                wext_sb[li] = []
                ws_sb[li] = []
                for ft in range(nft):
                    fr = min(P, fin - ft * P)
                    wt = persist.tile([fr, cw2], dt.float32, tag=f"wext{li}_{ft}")
                    nc.sync.dma_start(wt[:], wext_dram[li][ft * P : ft * P + fr, :])
                    wext_sb[li].append(wt)
                    st = persist.tile([fr, h], dt.float32, tag=f"ws{li}_{ft}")
                    nc.sync.dma_start(st[:], ws_dram[li][ft * P : ft * P + fr, :])
                    ws_sb[li].append(st)

            # ---------------- L1 own activations from input
            xTown_cur = []
            for ft in range(2):
                to = xTown_pool.tile([P, ROWS], dt.float32, tag="xTown")
                nc.sync.dma_start(to[:], x0T_own[ft * P : (ft + 1) * P, :])
                xTown_cur.append(to)

            def build_mask(chunks):
                # transpose adj rows -> additive maskT (bf16).
                CH = 1024
                for c0 in chunks:
                    for ib in range(ROWS // P):
                        stage_i = work.tile([P, CH], dt.int32, tag="stage_i")
                        # ACT's DGE queue: keeps this bulk stream from blocking
                        # the latency-critical gather/unpack DMAs on SP's queue
                        nc.scalar.dma_start(
                            stage_i[:], adjrows[ib * P : (ib + 1) * P, c0 : c0 + CH]
                        )
                        stage_b = work.tile([P, CH], dt.bfloat16, tag="stage_b")
                        nc.gpsimd.tensor_copy(stage_b[:], stage_i[:])
                        for k in range(CH // P):
                            jt = (c0 + k * P) // P
                            tps = trps.tile([P, P], dt.bfloat16, tag="tps")
                            nc.tensor.transpose(
                                tps[:], stage_b[:, k * P : (k + 1) * P], ident_sb[:]
                            )
                            # adj -> additive mask: adj*(-NEG) + NEG
                            nc.vector.tensor_scalar(
                                maskT[:, jt, ib * P : (ib + 1) * P],
                                tps[:],
                                -NEG,
                                NEG,
                                mybir.AluOpType.mult,
                                mybir.AluOpType.add,
                            )

            for li, (fin, fout, h, concat, elu, G) in enumerate(CFG, start=1):
                dh = fout // h if concat else fout
                dh1 = dh + 1
                cw2 = h * dh1 + h
                nft = (fin + P - 1) // P
                is_last = li == len(CFG)
                groups = _groups(h, G)

                # ---- own-block [Wh+ones (group-major) | all d cols].
                # The tiny d-only AllGather launches FIRST (scores gate on it);
                # vals AllGathers follow per group (matmuls gate on those).
                pw_all = ownp.tile([P, 4, cw2], dt.float32r, tag="own_sb")
                for k in range(4):
                    pw = whps.tile([P, cw2], dt.float32, tag="pw")
                    for ft in range(nft):
                        fr = min(P, fin - ft * P)
                        nc.tensor.matmul(
                            pw[:],
                            xTown_cur[ft][0:fr, k * P : (k + 1) * P],
                            wext_sb[li][ft][:],
                            start=(ft == 0),
                            stop=(ft == nft - 1),
                        )
                    nc.scalar.copy(pw_all[:, k, :], pw[:])
                # full-width gathers split along own-row halves: every
                # head's vals+d for rows k01 land in half the wire time;
                # the k23 half's lag is covered by k01 score work
                vals = pw_all[:, :, 0 : h * dh1].rearrange(
                    "p k (a b) -> p k a b", a=h
                )
                nc.scalar.copy(
                    vals[:, :, :, dh : dh + 1],
                    ones_blk[:, 0 : 4 * h].rearrange(
                        "p (k a b) -> p k a b", k=4, a=h
                    ),
                )
                halves = [(0, 4)]  # one gather per layer: real HW charges far more per collective than the cost model
                ag_halves = []
                for hi_, (k0, k1) in enumerate(halves):
                    nk = k1 - k0
                    ag_in = drampool.tile(
                        [nk * P, cw2],
                        dt.float32r,
                        tag=f"agin{li}h{hi_}",
                        name=f"agi{hi_}",
                    )
                    ag_out = drampool.tile(
                        [NCORE, nk * P, cw2],
                        dt.float32r,
                        tag=f"agout{li}h{hi_}",
                        addr_space="Shared",
                        name=f"ago{hi_}",
                    )
                    nc.sync.dma_start(
                        ag_in.rearrange("(k p) c -> p k c", p=P),
                        pw_all[:, k0:k1, :],
                    )
                    nc.gpsimd.collective_compute(
                        "AllGather",
                        mybir.AluOpType.bypass,
                        replica_groups=[list(range(NCORE))],
                        ins=[ag_in.opt()],
                        outs=[ag_out.opt()],
                    )
                    ag_halves.append(ag_out)

                # ---- unpack: one DMA per (half, k) into jt-strided slices
                whrow = layerbuf.tile(
                    [P, JT, cw2], dt.float32r, tag="whrow0", name="wr0"
                )
                wr0_4 = whrow.rearrange("p (r k) c -> p r k c", k=4)
                for hi_, (k0, k1) in enumerate(halves):
                    for k2 in range(k1 - k0):
                        nc.sync.dma_start(
                            wr0_4[:, :, k0 + k2, :],
                            ag_halves[hi_].rearrange(
                                "r (k p) c -> p r k c", p=P
                            )[:, :, k2, :],
                        )
                whrow_g = [whrow] * len(groups)
                d_all = whrow.rearrange("p j c -> p j c")[
                    :, :, h * dh1 : cw2
                ].bitcast(dt.float32)
                d_g = [d_all[:, :, gs[0] : gs[0] + len(gs)] for gs in groups]

                mask_chunks = list(range(0, N, 1024))

                def emit_sreps(hhs):
                    for hh in hhs:
                        ps_row = sps.tile([1, ROWS], dt.float32, tag="ps_row")
                        for ft in range(nft):
                            fr = min(P, fin - ft * P)
                            nc.tensor.matmul(
                                ps_row[:],
                                ws_sb[li][ft][:, hh : hh + 1],
                                xTown_cur[ft][0:fr, :],
                                start=(ft == 0),
                                stop=(ft == nft - 1),
                            )
                        s_row = small.tile([1, ROWS], dt.float32, tag="vec1")
                        nc.scalar.copy(s_row[:], ps_row[:])
                        srt = srep_pool.tile(
                            [P, ROWS], dt.float32, tag=f"srep{hh}", name=f"sr{hh}"
                        )
                        nc.gpsimd.partition_broadcast(srt[:], s_row[:])
                        sreps[hh] = srt

                sreps = {}
                if li == 1:
                    build_mask(mask_chunks[0:1])
                    mask_built = {0}
                    emit_sreps(range(h))
                else:
                    mask_built = set(range(len(mask_chunks)))
                    emit_sreps(range(h))

                # ---- attention per head group
                xnext = xTown_pool.tile([fout, ROWS], dt.float32, tag="xTown")
                n_groups = len(groups)
                # j-tiles whose group-0 gather half landed first go first
                if len(halves) == 2:
                    jt_order = [j for j in range(JT) if j % 4 < 2] + [
                        j for j in range(JT) if j % 4 >= 2
                    ]
                else:
                    jt_order = list(range(JT))

                def emit_epi2(gi, gs, osbs):
                    last_g = gi == n_groups - 1
                    for k, hh in enumerate(gs):
                        dve_path = last_g and (k % 2 == 0)
                        o_sb = osbs[hh]
                        # compute engines can't read at partition offset dh
                        # (must be 0/32/64/96); DMA the denominator row down
                        r_in = small.tile(
                            [1, ROWS], dt.float32, tag="vec1i", name="r_in"
                        )
                        nc.sync.dma_start(r_in[:], o_sb[dh : dh + 1, :])
                        r_sb = small.tile(
                            [1, ROWS], dt.float32, tag="vec1", name="r_sb"
                        )
                        nc.vector.reciprocal(r_sb[:], r_in[:])
                        ohead = small.tile(
                            [dh, ROWS], dt.float32, tag="ohead", name="ohead"
                        )
                        if dve_path:
                            rps = trps.tile([dh, ROWS], dt.float32, tag="rps")
                            nc.tensor.matmul(
                                rps[:], ones_row[0:1, 0:dh], r_sb[:],
                                start=True, stop=True,
                            )
                            nc.vector.tensor_mul(ohead[:], o_sb[0:dh, :], rps[:])
                        else:
                            rrep = small.tile(
                                [dh, ROWS], dt.float32, tag="rrep", name="rrep"
                            )
                            nc.gpsimd.partition_broadcast(rrep[:], r_sb[:])
                            nc.gpsimd.tensor_mul(ohead[:], o_sb[0:dh, :], rrep[:])
                        if elu:
                            # elu(x) = max(x,0) - 1 + exp(min(x,0))
                            mmin = small.tile(
                                [dh, ROWS], dt.float32, tag="tmp1", name="mmin"
                            )
                            emin = small.tile(
                                [dh, ROWS], dt.float32, tag="tmp2", name="emin"
                            )
                            rmax = small.tile(
                                [dh, ROWS], dt.float32, tag="tmp1", name="rmax"
                            )
                            if dve_path:
                                nc.vector.tensor_scalar(
                                    mmin[:], ohead[:], 0.0, None, mybir.AluOpType.min
                                )
                                nc.scalar.activation(emin[:], mmin[:], AF.Exp)
                                nc.vector.tensor_scalar(
                                    rmax[:], ohead[:], 0.0, -1.0,
                                    mybir.AluOpType.max, mybir.AluOpType.add,
                                )
                                nc.vector.tensor_add(ohead[:], rmax[:], emin[:])
                            else:
                                nc.gpsimd.tensor_scalar(
                                    mmin[:], ohead[:], 0.0, None, mybir.AluOpType.min
                                )
                                nc.scalar.activation(emin[:], mmin[:], AF.Exp)
                                nc.gpsimd.tensor_scalar(
                                    rmax[:], ohead[:], 0.0, -1.0,
                                    mybir.AluOpType.max, mybir.AluOpType.add,
                                )
                                nc.gpsimd.tensor_add(ohead[:], rmax[:], emin[:])
                        nc.sync.dma_start(xnext[hh * dh : (hh + 1) * dh, :], ohead[:])

                pending_epi = None
                att_accs = []
                for gi, gs in enumerate(groups):
                    ng = len(gs)
                    last_group = gi == n_groups - 1
                    att_acc = {}
                    for hh in gs:
                        att_acc[hh] = attps.tile(
                            [dh1, ROWS], dt.float32, tag="att", name=f"att{hh}"
                        )
                    att_accs.append(att_acc)
                    for jx, jt in enumerate(jt_order):
                        if jx == 8 and pending_epi is not None:
                            emit_epi2(*pending_epi)
                            pending_epi = None
                        ci = jt // 8
                        if ci not in mask_built:
                            build_mask(mask_chunks[ci : ci + 1])
                            mask_built.add(ci)
                        l_jt = work.tile([P, ng * ROWS], dt.float32, tag="l_jt")
                        for k, hh in enumerate(gs):
                            nc.vector._custom_dve(
                                LEAKY_BIAS_ADDMASK,
                                out=l_jt[:, k * ROWS : (k + 1) * ROWS],
                                in0=sreps[hh][:],
                                in1=maskT[:, jt, :],
                                s0=d_g[gi][:, jt, k : k + 1],
                                s1=ALPHA,
                            )
                        pool_p = pjts_pool if h == 1 else pjt_pool
                        p_jt = pool_p.tile([P, ng * ROWS], dt.float32r, tag="p_jt")
                        nc.scalar.activation(p_jt[:], l_jt[:], AF.Exp)
                        for k, hh in enumerate(gs):
                            nc.tensor.matmul(
                                att_acc[hh][:],
                                whrow_g[gi][:, jt, hh * dh1 : (hh + 1) * dh1],
                                p_jt[:, k * ROWS : (k + 1) * ROWS],
                                start=(jx == 0),
                                stop=(jx == JT - 1),
                            )
                    # epilogue phase 1: copy PSUM out now (frees att banks for
                    # the next group); phase 2 is deferred for non-last groups
                    # until 8 j-tiles into the next group's emission so its
                    # dependency waits never stall the in-order engine queues
                    osbs = {}
                    for k, hh in enumerate(gs):
                        o_sb = small.tile(
                            [dh1, ROWS], dt.float32, tag=f"o_sb{k}", name=f"os{k}"
                        )
                        nc.scalar.copy(o_sb[:], att_acc[hh][:])
                        osbs[hh] = o_sb
                    if last_group:
                        if pending_epi is not None:
                            emit_epi2(*pending_epi)
                            pending_epi = None
                        emit_epi2(gi, gs, osbs)
                    else:
                        pending_epi = (gi, gs, osbs)

                if debug_taps:
                    nc.sync.dma_start(dbg_x[li][:], xnext[:])
                if is_last:
                    psum_final = small.tile([fout, 1], dt.float32, tag="vec1f")
                    nc.vector.reduce_sum(
                        psum_final[:], xnext[:], axis=mybir.AxisListType.X
                    )
                    nc.sync.dma_start(pool_out[:], psum_final[:])
                else:
                    xTown_cur = [xnext]

    nc.finalize()
    return nc


_NC_CACHE = None
_last_in_maps = None


def kernel(**inputs):
    global _NC_CACHE
    node_features = np.asarray(inputs["node_features"], dtype=np.float32)
    adj = np.ascontiguousarray(np.asarray(inputs["adj_mat"], dtype=np.int32))
    fc_w = np.asarray(inputs["fc_w"], dtype=np.float32)
    fc_b = np.asarray(inputs["fc_b"], dtype=np.float32)

    x0T = node_features.T  # [256, N]

    wext = {}
    ws = {}
    for li, (fin, fout, h, concat, _elu, _g) in enumerate(CFG, start=1):
        dh = fout // h if concat else fout
        W = np.asarray(inputs[f"W{li}"], dtype=np.float32)  # [h, fin, dh]
        a_src = np.asarray(inputs[f"a_src{li}"], dtype=np.float32)  # [h, dh]
        a_dst = np.asarray(inputs[f"a_dst{li}"], dtype=np.float32)
        wd = np.einsum("hfd,hd->fh", W, a_dst).astype(np.float32)  # [fin, h]
        # augmented: per-head [W_h | zero(->ones)] blocks, then all d cols
        waug = np.zeros((fin, h * (dh + 1) + h), dtype=np.float32)
        for hh in range(h):
            waug[:, hh * (dh + 1) : hh * (dh + 1) + dh] = W[hh].reshape(fin, dh)
        waug[:, h * (dh + 1) :] = wd
        wext[li] = np.ascontiguousarray(waug)
        ws[li] = np.ascontiguousarray(
            np.einsum("hfd,hd->fh", W, a_src).astype(np.float32)
        )

    in_maps = []
    for c in range(NCORE):
        m = {
            "adjrows": np.ascontiguousarray(adj[c * ROWS : (c + 1) * ROWS, :]),
            "x0T_own": np.ascontiguousarray(x0T[:, c * ROWS : (c + 1) * ROWS]),
        }
        for li in range(1, 6):
            m[f"wext{li}"] = wext[li]
            m[f"ws{li}"] = ws[li]
        in_maps.append(m)

    if _NC_CACHE is None:
        _NC_CACHE = build_kernel()
    nc = _NC_CACHE
    global _last_in_maps
    _last_in_maps = in_maps

    res = run_bass_kernel_spmd(nc, in_maps, list(range(NCORE)))
    total = np.zeros((8,), dtype=np.float32)
    for c in range(NCORE):
        total += res.results[c]["pool_part"][:, 0]
    pooled = total / np.float32(N)
    out = pooled @ fc_w + fc_b
    return out.astype(np.float32)
